# revision 2
# baseline (speedup 1.0000x reference)
"""Trainium2 Bass kernel for nn_CubicSpline (embedding_lookup-style affine map).

Reference computes, for t in [0,1):
    w[n,i] = 1 - |t[n] - i|          (i = 0..62)
    out    = w @ cp[:63]             ([N,63] @ [63,128])

For t in [0,1] the triangular weights collapse algebraically:
    w[n,0] = 1 - t[n];   w[n,i] = t[n] + (1 - i)   (i >= 1)
so
    out[n,:] = t[n] * A + B
    A = sum_{i=1}^{62} cp[i] - cp[0]
    B = cp[0] + sum_{i=1}^{62} (1-i) * cp[i]

The device kernel therefore only needs to materialize a rank-1 affine map --
purely memory bound on the 512 MB fp32 output write.

Per-core layout (data-parallel over N across 8 cores, contiguous shards):
  * host packs the t-shard into 8 "phase" rows plus a ones row:
        t_aug[j, q] = t_shard[8*q + j]  (j<8);  t_aug[8, q] = 1.0
  * each 1024-row output tile g is produced by one K=9 weight load
    (lhsT = t_aug[:, 128g:128g+128]) and two N=512 fp32 matmuls against
    constant block-diagonal rhs tiles holding A (per phase) and B (ones row),
    so PSUM directly holds t*A + B for 1024 consecutive output rows
    in [128 partitions x 1024] layout (partition q -> rows 8q..8q+7).
  * PSUM -> SBUF copy alternates between VectorE and ScalarE.
  * each SBUF tile DMAs out as one fully contiguous 512 KB HBM write.
"""

import os
import sys
from contextlib import ExitStack

for _p in ("/opt/trn_rl_repo", "/root/.axon_site/_ro/trn_rl_repo"):
    if os.path.isdir(_p) and _p not in sys.path:
        sys.path.insert(0, _p)

import numpy as np

import concourse.mybir as mybir
import concourse.tile as tile
from concourse import bacc
from concourse import bass_utils

N_TOTAL = 1_000_000
D = 128
NUM_CP = 64
N_CORES = 8

R = 8                    # output rows per partition per tile (= #phase rows)
K = R + 1                # contraction dim: 8 phases + ones row
TILE_ROWS = 128 * R      # 1024 rows per output tile
TILES = 123              # tiles per core
NPC = TILES * TILE_ROWS  # 125_952 rows per core
NPAD = N_CORES * NPC     # 1_007_616 padded rows total
QTOT = NPC // R          # 15_744 q-columns per core
T_DMA_CHUNKS = 4         # split the t_aug load so compute can start early

F32 = mybir.dt.float32


def build_body(tc, out_ap, t_aug_ap, rhs_ap, tiles, qtot):
    """Tile-framework kernel body (shared by the real build and sim tests)."""
    nc = tc.nc
    # [tiles, 128, 1024] view of the output: tile g / partition q / free (w,d)
    # maps to row 1024g + 8q + w, col d -> fully contiguous 512KB per tile.
    out_t = out_ap.rearrange("(g q w) d -> g q (w d)", q=128, w=R)

    with ExitStack() as ctx:
        tpool = ctx.enter_context(tc.tile_pool(name="tpool", bufs=1))
        cpool = ctx.enter_context(tc.tile_pool(name="cpool", bufs=1))
        opool = ctx.enter_context(tc.tile_pool(name="opool", bufs=6))
        ppool = ctx.enter_context(tc.tile_pool(name="ppool", bufs=4, space="PSUM"))

        # rhs consts go out on the ACT HWDGE ring so they land immediately
        # (not queued behind the t_aug chunks on the SP ring).
        rhs_sb = cpool.tile([K, 2 * 512], F32)
        nc.scalar.dma_start(rhs_sb[:, 0:512], rhs_ap[0])
        nc.scalar.dma_start(rhs_sb[:, 512:1024], rhs_ap[1])

        t_sb = tpool.tile([K, qtot], F32)
        chunk = qtot // T_DMA_CHUNKS
        assert qtot % T_DMA_CHUNKS == 0
        for c in range(T_DMA_CHUNKS):
            sl = slice(c * chunk, (c + 1) * chunk)
            nc.sync.dma_start(t_sb[:, sl], t_aug_ap[:, sl])

        # Output DMAs rotate across the three descriptor-generation paths
        # (SP-HWDGE, ACT-HWDGE, gpsimd-SWDGE). Each path's ~2us completion
        # stall serializes only its own ring; rotating lets the 16 SDMA
        # engines stream another ring's packets during the stall.
        out_rings = [nc.sync, nc.scalar, nc.gpsimd]

        for g in range(tiles):
            psum = ppool.tile([128, 1024], F32, name="psum")
            lhsT = t_sb[:, g * 128 : (g + 1) * 128]
            nc.tensor.matmul(
                psum[:, 0:512], lhsT, rhs_sb[:, 0:512], start=True, stop=True
            )
            nc.tensor.matmul(
                psum[:, 512:1024], lhsT, rhs_sb[:, 512:1024], start=True, stop=True
            )
            ob = opool.tile([128, 1024], F32, name="ob")
            if g % 2 == 0:
                nc.vector.tensor_copy(ob[:], psum[:])
            else:
                nc.scalar.copy(ob[:], psum[:])
            out_rings[g % 3].dma_start(out_t[g], ob[:])


def build_nc(tiles=TILES):
    qtot = tiles * TILE_ROWS // R
    nc = bacc.Bacc(
        "TRN2", target_bir_lowering=False, debug=False, num_devices=N_CORES
    )
    t_aug = nc.dram_tensor("t_aug", [K, qtot], F32, kind="ExternalInput").ap()
    rhs_c = nc.dram_tensor("rhs_c", [2, K, 512], F32, kind="ExternalInput").ap()
    out = nc.dram_tensor("out", [tiles * TILE_ROWS, D], F32, kind="ExternalOutput").ap()
    with tile.TileContext(nc) as tc:
        build_body(tc, out, t_aug, rhs_c, tiles, qtot)
    nc.compile()
    return nc


def affine_consts(control_points):
    """A, B (float32 [128] each) with the reduction done in float64."""
    cp = np.asarray(control_points, dtype=np.float64)
    A = cp[1 : NUM_CP - 1].sum(axis=0) - cp[0]
    i = np.arange(1, NUM_CP - 1, dtype=np.float64)
    B = cp[0] + ((1.0 - i)[:, None] * cp[1 : NUM_CP - 1]).sum(axis=0)
    return A.astype(np.float32), B.astype(np.float32)


def make_rhs(A, B):
    """Constant rhs tiles [2, K, 512]: block-diagonal A per phase + B row."""
    rhs = np.zeros((2, K, 512), np.float32)
    for s in range(2):
        for m in range(4):
            rhs[s, m + 4 * s, 128 * m : 128 * (m + 1)] = A
            rhs[s, R, 128 * m : 128 * (m + 1)] = B
    return rhs


def make_t_aug(t_shard):
    """[K, QTOT]: 8 phase-deinterleaved rows of the shard + a ones row."""
    qtot = t_shard.shape[0] // R
    phases = t_shard.reshape(qtot, R).T  # [8, qtot], phases[j, q] = t[8q+j]
    return np.ascontiguousarray(
        np.concatenate([phases, np.ones((1, qtot), np.float32)], axis=0)
    )


_NC_CACHE = {}


def _get_nc():
    if "nc" not in _NC_CACHE:
        _NC_CACHE["nc"] = build_nc()
    return _NC_CACHE["nc"]


def prepare_in_maps(t, control_points):
    t = np.asarray(t, dtype=np.float32)
    A, B = affine_consts(control_points)
    rhs = make_rhs(A, B)
    t_clipped = np.clip(t, 0.0, 1.0)
    tpad = np.zeros(NPAD, np.float32)
    tpad[: t.shape[0]] = t_clipped
    shards = tpad.reshape(N_CORES, NPC)
    return [
        {"t_aug": make_t_aug(shards[c]), "rhs_c": rhs} for c in range(N_CORES)
    ]


def kernel(t, control_points):
    t = np.asarray(t)
    assert t.shape == (N_TOTAL,), t.shape
    nc = _get_nc()
    in_maps = prepare_in_maps(t, control_points)
    res = bass_utils.run_bass_kernel_spmd(
        nc, in_maps, core_ids=list(range(N_CORES))
    )
    full = np.concatenate([res.results[c]["out"] for c in range(N_CORES)], axis=0)
    return np.ascontiguousarray(full[:N_TOTAL]).astype(np.float32, copy=False)


if __name__ == "__main__":
    t = np.random.default_rng(0).random(N_TOTAL, dtype=np.float32)
    cp = np.random.default_rng(1).normal(size=(NUM_CP, D)).astype(np.float32)
    out = kernel(t, cp)
    A, B = affine_consts(cp)
    expect = t[:, None] * A[None, :] + B[None, :]
    err = np.abs(out - expect).max() / (np.abs(expect).max() + 1e-9)
    print("self-check max rel err:", err)


# revision 8
# speedup vs baseline: 2.1451x; 2.1451x over previous
"""Trainium2 Bass kernel for nn_CubicSpline (embedding_lookup-style affine map).

Reference computes, for t in [0,1):
    w[n,i] = 1 - |t[n] - i|          (i = 0..62)
    out    = w @ cp[:63]             ([N,63] @ [63,128])

For t in [0,1] the triangular weights collapse algebraically:
    w[n,0] = 1 - t[n];   w[n,i] = t[n] + (1 - i)   (i >= 1)
so
    out[n,:] = t[n] * A + B
    A = sum_{i=1}^{62} cp[i] - cp[0]
    B = cp[0] + sum_{i=1}^{62} (1-i) * cp[i]

The device kernel therefore only needs to materialize a rank-1 affine map --
purely memory bound on the 512 MB fp32 output write.

Per-core layout (data-parallel over N across 8 cores, contiguous shards):
  * host packs the t-shard into 8 "phase" rows plus a ones row:
        t_aug[j, q] = t_shard[8*q + j]  (j<8);  t_aug[8, q] = 1.0
  * each 1024-row output tile g is produced by one K=9 weight load
    (lhsT = t_aug[:, 128g:128g+128]) and two N=512 fp32 matmuls against
    constant block-diagonal rhs tiles holding A (per phase) and B (ones row),
    so PSUM directly holds t*A + B for 1024 consecutive output rows
    in [128 partitions x 1024] layout (partition q -> rows 8q..8q+7).
  * PSUM -> SBUF copy alternates between VectorE and ScalarE.
  * each SBUF tile DMAs out as one fully contiguous 512 KB HBM write.
"""

import os
import sys
from contextlib import ExitStack

for _p in ("/opt/trn_rl_repo", "/root/.axon_site/_ro/trn_rl_repo"):
    if os.path.isdir(_p) and _p not in sys.path:
        sys.path.insert(0, _p)

import ml_dtypes
import numpy as np

import concourse.mybir as mybir
import concourse.tile as tile
from concourse import bacc
from concourse import bass_utils

N_TOTAL = 1_000_000
D = 128
NUM_CP = 64
N_CORES = 8

R = 8                    # output rows per partition per tile (= #phase rows)
# Contraction rows (all bf16; PSUM accumulates fp32):
#   rows 0-7   : t_hi phases   x A_hi diag
#   rows 8-15  : t_lo phases   x A_hi diag
#   rows 16-23 : t_hi phases   x A_lo diag
#   row  24    : ones          x B_hi
#   row  25    : ones          x B_lo
# -> t*A + B to ~1e-6 rel (only t_lo*A_lo dropped). bf16 operands avoid the
# PE's fp32 HI/LO double-pass (2x matmul cost) and enable fast weight load.
K = 3 * R + 2            # 26
TILE_ROWS = 128 * R      # 1024 rows per output tile
TILES = 123              # tiles per core
NPC = TILES * TILE_ROWS  # 125_952 rows per core
NPAD = N_CORES * NPC     # 1_007_616 padded rows total
QTOT = NPC // R          # 15_744 q-columns per core
T_DMA_CHUNKS = 4         # split the t_aug load so compute can start early

F32 = mybir.dt.float32
BF16 = mybir.dt.bfloat16
NPBF16 = ml_dtypes.bfloat16


def build_body(tc, out_ap, t_aug_ap, rhs_ap, tiles, qtot):
    """Tile-framework kernel body (shared by the real build and sim tests)."""
    nc = tc.nc
    # [tiles, 128, 1024] view of the output: tile g / partition q / free (w,d)
    # maps to row 1024g + 8q + w, col d -> fully contiguous 512KB per tile.
    out_t = out_ap.rearrange("(g q w) d -> g q (w d)", q=128, w=R)

    with ExitStack() as ctx:
        tpool = ctx.enter_context(tc.tile_pool(name="tpool", bufs=1))
        cpool = ctx.enter_context(tc.tile_pool(name="cpool", bufs=1))
        opool = ctx.enter_context(tc.tile_pool(name="opool", bufs=6))
        ppool = ctx.enter_context(tc.tile_pool(name="ppool", bufs=4, space="PSUM"))

        # rhs consts go out on the ACT HWDGE ring so they land immediately
        # (not queued behind the t_aug chunks on the SP ring).
        rhs_sb = cpool.tile([K, 2 * 512], BF16)
        nc.scalar.dma_start(rhs_sb[:, 0:512], rhs_ap[0])
        nc.scalar.dma_start(rhs_sb[:, 512:1024], rhs_ap[1])

        t_sb = tpool.tile([K, qtot], BF16)
        chunk = qtot // T_DMA_CHUNKS
        assert qtot % T_DMA_CHUNKS == 0
        for c in range(T_DMA_CHUNKS):
            sl = slice(c * chunk, (c + 1) * chunk)
            nc.sync.dma_start(t_sb[:, sl], t_aug_ap[:, sl])

        # Output DMAs rotate across the three descriptor-generation paths
        # (SP-HWDGE, ACT-HWDGE, gpsimd-SWDGE). Each path's ~2us completion
        # stall serializes only its own ring; rotating lets the 16 SDMA
        # engines stream another ring's packets during the stall.
        out_rings = [nc.sync, nc.scalar, nc.gpsimd]

        for g in range(tiles):
            psum = ppool.tile([128, 1024], F32, name="psum")
            lhsT = t_sb[:, g * 128 : (g + 1) * 128]
            nc.tensor.matmul(
                psum[:, 0:512], lhsT, rhs_sb[:, 0:512], start=True, stop=True
            )
            nc.tensor.matmul(
                psum[:, 512:1024], lhsT, rhs_sb[:, 512:1024], start=True, stop=True
            )
            ob = opool.tile([128, 1024], F32, name="ob")
            if g % 2 == 0:
                nc.vector.tensor_copy(ob[:], psum[:])
            else:
                nc.scalar.copy(ob[:], psum[:])
            out_rings[g % 3].dma_start(out_t[g], ob[:])


def build_nc(tiles=TILES):
    qtot = tiles * TILE_ROWS // R
    nc = bacc.Bacc(
        "TRN2", target_bir_lowering=False, debug=False, num_devices=N_CORES
    )
    t_aug = nc.dram_tensor("t_aug", [K, qtot], BF16, kind="ExternalInput").ap()
    rhs_c = nc.dram_tensor("rhs_c", [2, K, 512], BF16, kind="ExternalInput").ap()
    out = nc.dram_tensor("out", [tiles * TILE_ROWS, D], F32, kind="ExternalOutput").ap()
    with tile.TileContext(nc) as tc:
        build_body(tc, out, t_aug, rhs_c, tiles, qtot)
    nc.compile()
    return nc


def _split_bf16(x64):
    """hi/lo bf16 split of a float64 array: hi + lo ~= x to ~2^-17 rel."""
    hi = x64.astype(NPBF16)
    lo = (x64 - hi.astype(np.float64)).astype(NPBF16)
    return hi, lo


def affine_consts(control_points):
    """A, B ([128] float64) of the collapsed affine map out = t*A + B."""
    cp = np.asarray(control_points, dtype=np.float64)
    A = cp[1 : NUM_CP - 1].sum(axis=0) - cp[0]
    i = np.arange(1, NUM_CP - 1, dtype=np.float64)
    B = cp[0] + ((1.0 - i)[:, None] * cp[1 : NUM_CP - 1]).sum(axis=0)
    return A, B


def make_rhs(A, B):
    """Constant rhs tiles [2, K, 512] bf16 (see row layout at top)."""
    A_hi, A_lo = _split_bf16(A)
    B_hi, B_lo = _split_bf16(B)
    rhs = np.zeros((2, K, 512), NPBF16)
    for s in range(2):
        for m in range(4):
            j = m + 4 * s
            sl = slice(128 * m, 128 * (m + 1))
            rhs[s, j, sl] = A_hi
            rhs[s, 8 + j, sl] = A_hi
            rhs[s, 16 + j, sl] = A_lo
            rhs[s, 24, sl] = B_hi
            rhs[s, 25, sl] = B_lo
    return rhs


def make_t_aug(t_shard):
    """[K, QTOT] bf16: t_hi, t_lo, t_hi phase rows + two ones rows."""
    qtot = t_shard.shape[0] // R
    t64 = t_shard.astype(np.float64)
    t_hi, t_lo = _split_bf16(t64)
    ph_hi = t_hi.reshape(qtot, R).T  # [8, qtot], ph[j, q] = t[8q+j]
    ph_lo = t_lo.reshape(qtot, R).T
    ones = np.ones((2, qtot), NPBF16)
    return np.ascontiguousarray(
        np.concatenate([ph_hi, ph_lo, ph_hi, ones], axis=0)
    )


_NC_CACHE = {}


def _get_nc():
    if "nc" not in _NC_CACHE:
        _NC_CACHE["nc"] = build_nc()
    return _NC_CACHE["nc"]


def prepare_in_maps(t, control_points):
    t = np.asarray(t, dtype=np.float32)
    A, B = affine_consts(control_points)
    rhs = make_rhs(A, B)
    t_clipped = np.clip(t, 0.0, 1.0)
    tpad = np.zeros(NPAD, np.float32)
    tpad[: t.shape[0]] = t_clipped
    shards = tpad.reshape(N_CORES, NPC)
    return [
        {"t_aug": make_t_aug(shards[c]), "rhs_c": rhs} for c in range(N_CORES)
    ]


def kernel(t, control_points):
    t = np.asarray(t)
    assert t.shape == (N_TOTAL,), t.shape
    nc = _get_nc()
    in_maps = prepare_in_maps(t, control_points)
    res = bass_utils.run_bass_kernel_spmd(
        nc, in_maps, core_ids=list(range(N_CORES))
    )
    full = np.concatenate([res.results[c]["out"] for c in range(N_CORES)], axis=0)
    return np.ascontiguousarray(full[:N_TOTAL]).astype(np.float32, copy=False)


if __name__ == "__main__":
    t = np.random.default_rng(0).random(N_TOTAL, dtype=np.float32)
    cp = np.random.default_rng(1).normal(size=(NUM_CP, D)).astype(np.float32)
    out = kernel(t, cp)
    A, B = affine_consts(cp)
    expect = t.astype(np.float64)[:, None] * A[None, :] + B[None, :]
    err = np.abs(out - expect).max() / (np.abs(expect).max() + 1e-9)
    print("self-check max rel err:", err)


# revision 10
# speedup vs baseline: 2.1746x; 1.0138x over previous
"""Trainium2 Bass kernel for nn_CubicSpline (embedding_lookup-style affine map).

Reference computes, for t in [0,1):
    w[n,i] = 1 - |t[n] - i|          (i = 0..62)
    out    = w @ cp[:63]             ([N,63] @ [63,128])

For t in [0,1] the triangular weights collapse algebraically:
    w[n,0] = 1 - t[n];   w[n,i] = t[n] + (1 - i)   (i >= 1)
so
    out[n,:] = t[n] * A + B
    A = sum_{i=1}^{62} cp[i] - cp[0]
    B = cp[0] + sum_{i=1}^{62} (1-i) * cp[i]

The device kernel therefore only needs to materialize a rank-1 affine map --
purely memory bound on the 512 MB fp32 output write.

Per-core layout (data-parallel over N across 8 cores, contiguous shards):
  * host packs the t-shard into 8 "phase" rows plus a ones row:
        t_aug[j, q] = t_shard[8*q + j]  (j<8);  t_aug[8, q] = 1.0
  * each 1024-row output tile g is produced by one K=9 weight load
    (lhsT = t_aug[:, 128g:128g+128]) and two N=512 fp32 matmuls against
    constant block-diagonal rhs tiles holding A (per phase) and B (ones row),
    so PSUM directly holds t*A + B for 1024 consecutive output rows
    in [128 partitions x 1024] layout (partition q -> rows 8q..8q+7).
  * PSUM -> SBUF copy alternates between VectorE and ScalarE.
  * each SBUF tile DMAs out as one fully contiguous 512 KB HBM write.
"""

import os
import sys
from contextlib import ExitStack

for _p in ("/opt/trn_rl_repo", "/root/.axon_site/_ro/trn_rl_repo"):
    if os.path.isdir(_p) and _p not in sys.path:
        sys.path.insert(0, _p)

import ml_dtypes
import numpy as np

import concourse.mybir as mybir
import concourse.tile as tile
from concourse import bacc
from concourse import bass_utils

N_TOTAL = 1_000_000
D = 128
NUM_CP = 64
N_CORES = 8

R = 8                    # output rows per partition per tile (= #phase rows)
# Contraction rows (all bf16; PSUM accumulates fp32):
#   rows 0-7   : t_hi phases   x A_hi diag
#   rows 8-15  : t_lo phases   x A_hi diag
#   rows 16-23 : t_hi phases   x A_lo diag
#   row  24    : ones          x B_hi
#   row  25    : ones          x B_lo
# -> t*A + B to ~1e-6 rel (only t_lo*A_lo dropped). bf16 operands avoid the
# PE's fp32 HI/LO double-pass (2x matmul cost) and enable fast weight load.
K = 3 * R + 2            # 26
TILE_ROWS = 128 * R      # 1024 rows per output tile
TILES = 123              # tiles per core
NPC = TILES * TILE_ROWS  # 125_952 rows per core
NPAD = N_CORES * NPC     # 1_007_616 padded rows total
QTOT = NPC // R          # 15_744 q-columns per core
T_DMA_CHUNKS = 3         # independent t tiles, one per DMA ring (123 = 3*41)

F32 = mybir.dt.float32
BF16 = mybir.dt.bfloat16
NPBF16 = ml_dtypes.bfloat16


def build_body(tc, out_ap, t_aug_ap, rhs_ap, tiles, qtot):
    """Tile-framework kernel body (shared by the real build and sim tests)."""
    nc = tc.nc
    # [tiles, 128, 1024] view of the output: tile g / partition q / free (w,d)
    # maps to row 1024g + 8q + w, col d -> fully contiguous 512KB per tile.
    out_t = out_ap.rearrange("(g q w) d -> g q (w d)", q=128, w=R)

    with ExitStack() as ctx:
        tpool = ctx.enter_context(tc.tile_pool(name="tpool", bufs=1))
        cpool = ctx.enter_context(tc.tile_pool(name="cpool", bufs=1))
        opool = ctx.enter_context(tc.tile_pool(name="opool", bufs=6))
        ppool = ctx.enter_context(tc.tile_pool(name="ppool", bufs=4, space="PSUM"))

        # rhs consts go out on the ACT HWDGE ring so they land immediately
        # (not queued behind the t_aug chunks on the SP ring).
        rhs_sb = cpool.tile([K, 2 * 512], BF16)
        nc.scalar.dma_start(rhs_sb[:, 0:512], rhs_ap[0])
        nc.scalar.dma_start(rhs_sb[:, 512:1024], rhs_ap[1])

        # Output DMAs rotate across the three descriptor-generation paths
        # (SP-HWDGE, ACT-HWDGE, gpsimd-SWDGE). Each path's ~2us completion
        # stall serializes only its own ring; rotating lets the 16 SDMA
        # engines stream another ring's packets during the stall.
        out_rings = [nc.sync, nc.scalar, nc.gpsimd]

        # t_aug loads as independent tiles, one per ring, all in parallel,
        # so the first matmul only waits for its own chunk (~2-3us).
        chunk = qtot // T_DMA_CHUNKS
        assert qtot % T_DMA_CHUNKS == 0 and chunk % 128 == 0
        t_tiles = []
        for c in range(T_DMA_CHUNKS):
            tt = tpool.tile([K, chunk], BF16, name=f"tch{c}", tag=f"tch{c}")
            out_rings[c % 3].dma_start(
                tt[:], t_aug_ap[:, c * chunk : (c + 1) * chunk]
            )
            t_tiles.append(tt)
        groups_per_chunk = chunk // 128

        for g in range(tiles):
            psum = ppool.tile([128, 1024], F32, name="psum")
            ci, co = divmod(g, groups_per_chunk)
            lhsT = t_tiles[ci][:, co * 128 : (co + 1) * 128]
            nc.tensor.matmul(
                psum[:, 0:512], lhsT, rhs_sb[:, 0:512], start=True, stop=True
            )
            nc.tensor.matmul(
                psum[:, 512:1024], lhsT, rhs_sb[:, 512:1024], start=True, stop=True
            )
            ob = opool.tile([128, 1024], F32, name="ob")
            if g % 2 == 0:
                nc.vector.tensor_copy(ob[:], psum[:])
            else:
                nc.scalar.copy(ob[:], psum[:])
            out_rings[g % 3].dma_start(out_t[g], ob[:])


def build_nc(tiles=TILES):
    qtot = tiles * TILE_ROWS // R
    nc = bacc.Bacc(
        "TRN2", target_bir_lowering=False, debug=False, num_devices=N_CORES
    )
    t_aug = nc.dram_tensor("t_aug", [K, qtot], BF16, kind="ExternalInput").ap()
    rhs_c = nc.dram_tensor("rhs_c", [2, K, 512], BF16, kind="ExternalInput").ap()
    out = nc.dram_tensor("out", [tiles * TILE_ROWS, D], F32, kind="ExternalOutput").ap()
    with tile.TileContext(nc) as tc:
        build_body(tc, out, t_aug, rhs_c, tiles, qtot)
    nc.compile()
    return nc


def _split_bf16(x64):
    """hi/lo bf16 split of a float64 array: hi + lo ~= x to ~2^-17 rel."""
    hi = x64.astype(NPBF16)
    lo = (x64 - hi.astype(np.float64)).astype(NPBF16)
    return hi, lo


def affine_consts(control_points):
    """A, B ([128] float64) of the collapsed affine map out = t*A + B."""
    cp = np.asarray(control_points, dtype=np.float64)
    A = cp[1 : NUM_CP - 1].sum(axis=0) - cp[0]
    i = np.arange(1, NUM_CP - 1, dtype=np.float64)
    B = cp[0] + ((1.0 - i)[:, None] * cp[1 : NUM_CP - 1]).sum(axis=0)
    return A, B


def make_rhs(A, B):
    """Constant rhs tiles [2, K, 512] bf16 (see row layout at top)."""
    A_hi, A_lo = _split_bf16(A)
    B_hi, B_lo = _split_bf16(B)
    rhs = np.zeros((2, K, 512), NPBF16)
    for s in range(2):
        for m in range(4):
            j = m + 4 * s
            sl = slice(128 * m, 128 * (m + 1))
            rhs[s, j, sl] = A_hi
            rhs[s, 8 + j, sl] = A_hi
            rhs[s, 16 + j, sl] = A_lo
            rhs[s, 24, sl] = B_hi
            rhs[s, 25, sl] = B_lo
    return rhs


def make_t_aug(t_shard):
    """[K, QTOT] bf16: t_hi, t_lo, t_hi phase rows + two ones rows."""
    qtot = t_shard.shape[0] // R
    t64 = t_shard.astype(np.float64)
    t_hi, t_lo = _split_bf16(t64)
    ph_hi = t_hi.reshape(qtot, R).T  # [8, qtot], ph[j, q] = t[8q+j]
    ph_lo = t_lo.reshape(qtot, R).T
    ones = np.ones((2, qtot), NPBF16)
    return np.ascontiguousarray(
        np.concatenate([ph_hi, ph_lo, ph_hi, ones], axis=0)
    )


_NC_CACHE = {}


def _get_nc():
    if "nc" not in _NC_CACHE:
        _NC_CACHE["nc"] = build_nc()
    return _NC_CACHE["nc"]


def prepare_in_maps(t, control_points):
    t = np.asarray(t, dtype=np.float32)
    A, B = affine_consts(control_points)
    rhs = make_rhs(A, B)
    t_clipped = np.clip(t, 0.0, 1.0)
    tpad = np.zeros(NPAD, np.float32)
    tpad[: t.shape[0]] = t_clipped
    shards = tpad.reshape(N_CORES, NPC)
    return [
        {"t_aug": make_t_aug(shards[c]), "rhs_c": rhs} for c in range(N_CORES)
    ]


def kernel(t, control_points):
    t = np.asarray(t)
    assert t.shape == (N_TOTAL,), t.shape
    nc = _get_nc()
    in_maps = prepare_in_maps(t, control_points)
    res = bass_utils.run_bass_kernel_spmd(
        nc, in_maps, core_ids=list(range(N_CORES))
    )
    full = np.concatenate([res.results[c]["out"] for c in range(N_CORES)], axis=0)
    return np.ascontiguousarray(full[:N_TOTAL]).astype(np.float32, copy=False)


if __name__ == "__main__":
    t = np.random.default_rng(0).random(N_TOTAL, dtype=np.float32)
    cp = np.random.default_rng(1).normal(size=(NUM_CP, D)).astype(np.float32)
    out = kernel(t, cp)
    A, B = affine_consts(cp)
    expect = t.astype(np.float64)[:, None] * A[None, :] + B[None, :]
    err = np.abs(out - expect).max() / (np.abs(expect).max() + 1e-9)
    print("self-check max rel err:", err)


# revision 15
# speedup vs baseline: 2.1821x; 1.0034x over previous
"""Trainium2 Bass kernel for nn_CubicSpline (embedding_lookup-style affine map).

Reference computes, for t in [0,1):
    w[n,i] = 1 - |t[n] - i|          (i = 0..62)
    out    = w @ cp[:63]             ([N,63] @ [63,128])

For t in [0,1] the triangular weights collapse algebraically:
    w[n,0] = 1 - t[n];   w[n,i] = t[n] + (1 - i)   (i >= 1)
so
    out[n,:] = t[n] * A + B
    A = sum_{i=1}^{62} cp[i] - cp[0]
    B = cp[0] + sum_{i=1}^{62} (1-i) * cp[i]

The device kernel therefore only needs to materialize a rank-1 affine map --
purely memory bound on the 512 MB fp32 output write.

Per-core layout (data-parallel over N across 8 cores, contiguous shards):
  * host packs the t-shard into 8 "phase" rows plus a ones row:
        t_aug[j, q] = t_shard[8*q + j]  (j<8);  t_aug[8, q] = 1.0
  * each 1024-row output tile g is produced by one K=9 weight load
    (lhsT = t_aug[:, 128g:128g+128]) and two N=512 fp32 matmuls against
    constant block-diagonal rhs tiles holding A (per phase) and B (ones row),
    so PSUM directly holds t*A + B for 1024 consecutive output rows
    in [128 partitions x 1024] layout (partition q -> rows 8q..8q+7).
  * PSUM -> SBUF copy alternates between VectorE and ScalarE.
  * each SBUF tile DMAs out as one fully contiguous 512 KB HBM write.
"""

import os
import sys
from contextlib import ExitStack

for _p in ("/opt/trn_rl_repo", "/root/.axon_site/_ro/trn_rl_repo"):
    if os.path.isdir(_p) and _p not in sys.path:
        sys.path.insert(0, _p)

import ml_dtypes
import numpy as np

import concourse.mybir as mybir
import concourse.tile as tile
from concourse import bacc
from concourse import bass_utils

N_TOTAL = 1_000_000
D = 128
NUM_CP = 64
N_CORES = 8

R = 8                    # output rows per partition per tile (= #phase rows)
# Contraction rows (all bf16; PSUM accumulates fp32):
#   rows 0-7   : t_hi phases   x A_hi diag
#   rows 8-15  : t_lo phases   x A_hi diag
#   rows 16-23 : t_hi phases   x A_lo diag
#   row  24    : ones          x B_hi
#   row  25    : ones          x B_lo
# -> t*A + B to ~1e-6 rel (only t_lo*A_lo dropped). bf16 operands avoid the
# PE's fp32 HI/LO double-pass (2x matmul cost) and enable fast weight load.
K = 3 * R + 2            # 26
TILE_ROWS = 128 * R      # 1024 rows per output tile
TILES = 123              # tiles per core
NPC = TILES * TILE_ROWS  # 125_952 rows per core
NPAD = N_CORES * NPC     # 1_007_616 padded rows total
QTOT = NPC // R          # 15_744 q-columns per core
T_DMA_CHUNKS = 3         # independent t tiles, one per DMA ring (123 = 3*41)

F32 = mybir.dt.float32
BF16 = mybir.dt.bfloat16
NPBF16 = ml_dtypes.bfloat16


def build_body(tc, out_ap, t_aug_ap, rhs_ap, tiles, qtot):
    """Tile-framework kernel body (shared by the real build and sim tests)."""
    nc = tc.nc
    # [tiles, 128, 1024] view of the output: tile g / partition q / free (w,d)
    # maps to row 1024g + 8q + w, col d -> fully contiguous 512KB per tile.
    out_t = out_ap.rearrange("(g q w) d -> g q (w d)", q=128, w=R)

    with ExitStack() as ctx:
        tpool = ctx.enter_context(tc.tile_pool(name="tpool", bufs=1))
        cpool = ctx.enter_context(tc.tile_pool(name="cpool", bufs=1))
        opool = ctx.enter_context(tc.tile_pool(name="opool", bufs=6))
        ppool = ctx.enter_context(tc.tile_pool(name="ppool", bufs=4, space="PSUM"))

        # rhs consts go out on the ACT HWDGE ring so they land immediately
        # (not queued behind the t_aug chunks on the SP ring).
        rhs_sb = cpool.tile([K, 2 * 512], BF16)
        nc.scalar.dma_start(rhs_sb[:, 0:512], rhs_ap[0])
        nc.scalar.dma_start(rhs_sb[:, 512:1024], rhs_ap[1])

        # Output DMAs rotate across the three descriptor-generation paths
        # (SP-HWDGE, ACT-HWDGE, gpsimd-SWDGE). Each path's ~2us completion
        # stall serializes only its own ring; rotating lets the 16 SDMA
        # engines stream another ring's packets during the stall.
        out_rings = [nc.sync, nc.scalar, nc.gpsimd]

        # t_aug loads as independent tiles spread across the rings, all in
        # parallel. The first chunk is a single 128-col group so the first
        # matmul's dependency lands in ~1us; the rest follow concurrently.
        ngroups = qtot // 128
        nparts = min(T_DMA_CHUNKS, ngroups)
        base, extra = divmod(ngroups, nparts)
        bounds = [0]
        for c in range(nparts):
            take = base + (1 if c < extra else 0)
            bounds.append(bounds[-1] + take * 128)
        t_tiles = []
        for c in range(len(bounds) - 1):
            lo, hi = bounds[c], bounds[c + 1]
            tt = tpool.tile([K, hi - lo], BF16, name=f"tch{c}", tag=f"tch{c}")
            out_rings[c % 3].dma_start(tt[:], t_aug_ap[:, lo:hi])
            t_tiles.append(tt)

        def lhsT_for(g):
            col = g * 128
            for c in range(len(bounds) - 1):
                if col < bounds[c + 1]:
                    off = col - bounds[c]
                    return t_tiles[c][:, off : off + 128]
            raise AssertionError

        for g in range(tiles):
            psum = ppool.tile([128, 1024], F32, name="psum")
            lhsT = lhsT_for(g)
            nc.tensor.matmul(
                psum[:, 0:512], lhsT, rhs_sb[:, 0:512], start=True, stop=True
            )
            nc.tensor.matmul(
                psum[:, 512:1024], lhsT, rhs_sb[:, 512:1024], start=True, stop=True
            )
            ob = opool.tile([128, 1024], F32, name="ob")
            if g % 2 == 0:
                nc.vector.tensor_copy(ob[:], psum[:])
            else:
                nc.scalar.copy(ob[:], psum[:])
            out_rings[g % 3].dma_start(out_t[g], ob[:])


def build_nc(tiles=TILES):
    qtot = tiles * TILE_ROWS // R
    nc = bacc.Bacc(
        "TRN2", target_bir_lowering=False, debug=False, num_devices=N_CORES
    )
    t_aug = nc.dram_tensor("t_aug", [K, qtot], BF16, kind="ExternalInput").ap()
    rhs_c = nc.dram_tensor("rhs_c", [2, K, 512], BF16, kind="ExternalInput").ap()
    out = nc.dram_tensor("out", [tiles * TILE_ROWS, D], F32, kind="ExternalOutput").ap()
    with tile.TileContext(nc) as tc:
        build_body(tc, out, t_aug, rhs_c, tiles, qtot)
    nc.compile()
    return nc


def _split_bf16(x64):
    """hi/lo bf16 split of a float64 array: hi + lo ~= x to ~2^-17 rel."""
    hi = x64.astype(NPBF16)
    lo = (x64 - hi.astype(np.float64)).astype(NPBF16)
    return hi, lo


def affine_consts(control_points):
    """A, B ([128] float64) of the collapsed affine map out = t*A + B."""
    cp = np.asarray(control_points, dtype=np.float64)
    A = cp[1 : NUM_CP - 1].sum(axis=0) - cp[0]
    i = np.arange(1, NUM_CP - 1, dtype=np.float64)
    B = cp[0] + ((1.0 - i)[:, None] * cp[1 : NUM_CP - 1]).sum(axis=0)
    return A, B


def make_rhs(A, B):
    """Constant rhs tiles [2, K, 512] bf16 (see row layout at top)."""
    A_hi, A_lo = _split_bf16(A)
    B_hi, B_lo = _split_bf16(B)
    rhs = np.zeros((2, K, 512), NPBF16)
    for s in range(2):
        for m in range(4):
            j = m + 4 * s
            sl = slice(128 * m, 128 * (m + 1))
            rhs[s, j, sl] = A_hi
            rhs[s, 8 + j, sl] = A_hi
            rhs[s, 16 + j, sl] = A_lo
            rhs[s, 24, sl] = B_hi
            rhs[s, 25, sl] = B_lo
    return rhs


def make_t_aug(t_shard):
    """[K, QTOT] bf16: t_hi, t_lo, t_hi phase rows + two ones rows."""
    qtot = t_shard.shape[0] // R
    t64 = t_shard.astype(np.float64)
    t_hi, t_lo = _split_bf16(t64)
    ph_hi = t_hi.reshape(qtot, R).T  # [8, qtot], ph[j, q] = t[8q+j]
    ph_lo = t_lo.reshape(qtot, R).T
    ones = np.ones((2, qtot), NPBF16)
    return np.ascontiguousarray(
        np.concatenate([ph_hi, ph_lo, ph_hi, ones], axis=0)
    )


_NC_CACHE = {}


def _get_nc():
    if "nc" not in _NC_CACHE:
        _NC_CACHE["nc"] = build_nc()
    return _NC_CACHE["nc"]


def prepare_in_maps(t, control_points):
    t = np.asarray(t, dtype=np.float32)
    A, B = affine_consts(control_points)
    rhs = make_rhs(A, B)
    t_clipped = np.clip(t, 0.0, 1.0)
    tpad = np.zeros(NPAD, np.float32)
    tpad[: t.shape[0]] = t_clipped
    shards = tpad.reshape(N_CORES, NPC)
    return [
        {"t_aug": make_t_aug(shards[c]), "rhs_c": rhs} for c in range(N_CORES)
    ]


def kernel(t, control_points):
    t = np.asarray(t)
    assert t.shape == (N_TOTAL,), t.shape
    nc = _get_nc()
    in_maps = prepare_in_maps(t, control_points)
    res = bass_utils.run_bass_kernel_spmd(
        nc, in_maps, core_ids=list(range(N_CORES))
    )
    full = np.concatenate([res.results[c]["out"] for c in range(N_CORES)], axis=0)
    return np.ascontiguousarray(full[:N_TOTAL]).astype(np.float32, copy=False)


if __name__ == "__main__":
    t = np.random.default_rng(0).random(N_TOTAL, dtype=np.float32)
    cp = np.random.default_rng(1).normal(size=(NUM_CP, D)).astype(np.float32)
    out = kernel(t, cp)
    A, B = affine_consts(cp)
    expect = t.astype(np.float64)[:, None] * A[None, :] + B[None, :]
    err = np.abs(out - expect).max() / (np.abs(expect).max() + 1e-9)
    print("self-check max rel err:", err)
